# revision 17
# baseline (speedup 1.0000x reference)
"""Multi-head attention (N=4, L=2048, C=1024, H=16, D=64) on 8 TRN2 NeuronCores.

Sharding: core c -> batch n = c//2, head-group g = c%2 (8 heads each).
Each core computes its 8 heads' attention + the partial output projection
for batch n; the host sums the two partials per batch and adds the
constant bias term (b_out + b_v @ W_out).

Device-side layout (per core):
  xT   [C=1024, L=2048]  bf16 (x[n].T, host-transposed/cast)
  wqk  [C, 1024]         bf16 (W_in cols: 8 heads' q dims then k dims)
  wv   [C, 512]          bf16 (W_in cols: 8 heads' v dims)
  wo   [512, F=1024]     bf16 (W_out rows for the 8 heads)
  bqk  [128, 8]          f32  (q/k bias, partition-major per j-tile)
  y    [L, F]            f32  output partial

Pipeline (exp split across ACT+DVE paces the chunks; PE hides under it):
  - PE warm-up spin matmuls + exp-table preload run during the input DMA
    window so the HAM clock gate is released before real work starts; the
    first two q/k projection units are interleaved per c-tile (through the
    score PSUM banks) to keep the PE dense while the DMA streams
  - qT/kT = W^T @ xT (j on partitions), V = xT^T @ Wv (l on partitions)
  - scoresT[k, q] per head, row-tiled head pairs (K=64 -> rows 0-63 / 64-127)
  - exp: 11/16 k-tiles on ACT (scale=1/8 fused, fp32 PSUM -> bf16 SBUF),
    5/16 on DVE via Schraudolph (one fused multiply-add emitting the bf16
    bit pattern of 2^t as int16) so the two exp streams run concurrently
  - AV^T as col-tiled pairs into avD (head A rows 0:64, head B 64:128);
    row sums as M=64 ones-matmuls into avS with the same partition split,
    so 1/sum (bit-trick seed + one Newton step, whole-tile DVE ops) and
    the normalize multiply are single [128,512] instructions per chunk
  - qkT projections for the next pair and the final y projection are
    interleaved into the attention chunks as PE filler work; y copies
    alternate DVE/ACT and y DMAs alternate the gpsimd/sync queues
"""

import sys
from contextlib import ExitStack

import numpy as np

sys.path.insert(0, "/opt/trn_rl_repo")

import ml_dtypes

import concourse.bass as bass
import concourse.tile as tile
from concourse import bacc, mybir
from concourse.bass_utils import run_bass_kernel_spmd

BF16 = mybir.dt.bfloat16
F32 = mybir.dt.float32
I16 = mybir.dt.int16
I32 = mybir.dt.int32
FT = mybir.ActivationFunctionType
MULT = mybir.AluOpType.mult
ADD = mybir.AluOpType.add

N, L, C, H, D = 4, 2048, 1024, 16, 64
QKV = H * D  # 1024
F = 1024  # output feature dim
HG = 8  # heads per core
NCORES = 8
SCALE = float(D) ** -0.5  # 0.125

CT = C // 128  # 8 c-tiles
LT = L // 128  # 16 l-tiles
JQ = L // 512  # 4 q-chunks
KT = L // 128  # 16 k-tiles
NP = HG // 2  # 4 head pairs

# Softmax-exp engine split: k-tiles in DVE_KT evaluate exp on the vector
# engine via the Schraudolph bit trick (bf16 bit pattern of 2^t built with
# one fused multiply-add, written as int16), the rest on ACT. Spreading the
# DVE tiles through the chunk keeps both exp streams concurrent.
DVE_KT = frozenset({2, 5, 8, 11, 14})
SCH_A = SCALE * 128.0 / float(np.log(2.0))  # score -> bf16-bit scale
SCH_B = 16250.5  # centered exponent-bias constant
RCP_MAGIC = float(0x7EF30000)  # Newton seed: r0_bits = MAGIC - x_bits

# Globals for test harness introspection
TRACE = False
LAST_RESULTS = None


def _build_program() -> bass.Bass:
    nc = bacc.Bacc()

    xT_d = nc.declare_dram_parameter("xT", [C, L], BF16, isOutput=False)
    wqk_d = nc.declare_dram_parameter("wqk", [C, 1024], BF16, isOutput=False)
    wv_d = nc.declare_dram_parameter("wv", [C, 512], BF16, isOutput=False)
    wo_d = nc.declare_dram_parameter("wo", [512, F], BF16, isOutput=False)
    bqk_d = nc.declare_dram_parameter("bqk", [128, 8], F32, isOutput=False)
    y_d = nc.declare_dram_parameter("y", [L, F], BF16, isOutput=True)

    with tile.TileContext(nc) as tc, ExitStack() as ctx:
        const_pool = ctx.enter_context(tc.tile_pool(name="const", bufs=1))
        qk_pool = ctx.enter_context(tc.tile_pool(name="qkT", bufs=1))
        v_pool = ctx.enter_context(tc.tile_pool(name="V", bufs=1))
        outT_pool = ctx.enter_context(tc.tile_pool(name="outT", bufs=1))
        exp_pool = ctx.enter_context(tc.tile_pool(name="expT", bufs=2))
        r_pool = ctx.enter_context(tc.tile_pool(name="r", bufs=1))
        y_pool = ctx.enter_context(tc.tile_pool(name="y", bufs=2))
        wo_pool = ctx.enter_context(tc.tile_pool(name="wo", bufs=1))
        # PSUM: scores 2x2 banks + av 2 + proj 2 = 8 banks
        ps_s = ctx.enter_context(tc.tile_pool(name="ps_s", bufs=2, space="PSUM"))
        ps_av = ctx.enter_context(tc.tile_pool(name="ps_av", bufs=1, space="PSUM"))
        ps_proj = ctx.enter_context(tc.tile_pool(name="ps_proj", bufs=1, space="PSUM"))

        ones64 = const_pool.tile([128, 64], BF16)
        nc.vector.memset(ones64[:], 1.0)
        dummy = const_pool.tile([128, 1], F32)
        bqk_sb = const_pool.tile([128, 8], F32)
        nc.sync.dma_start(bqk_sb[:], bqk_d[:])
        wo_sb = wo_pool.tile([128, 4, F], BF16)
        # wo rides the gpsimd DMA queue (idle until the output DMAs start
        # ~300us in): it stops delaying the projection-critical xT/wqk
        # transfers at the head of the sync queue, and it has huge slack
        # (first consumer is the first y unit).
        nc.gpsimd.dma_start(wo_sb[:], wo_d.rearrange("(t p) f -> p t f", p=128))

        # qT/kT: [128, jt(8), jl(4), 512] ; jt 0-3 q dims, 4-7 k dims.
        qkT_sb = qk_pool.tile([128, 8, 4, 512], BF16)
        # V65: [128, lt(16), head(8), 65]; col 64 of each head block is ones
        # so the AV matmul's 65th output row accumulates the softmax sums.
        V65_sb = v_pool.tile([128, LT, HG, 65], BF16)
        nc.vector.memset(V65_sb[:, :, :, 64:65], 1.0)
        # outT: [128, pair(4), L] (partitions = 2 heads x 64 dims)
        outT_sb = outT_pool.tile([128, NP, L], BF16)

        # Exp-table preload on ACT (hides the ~1.3us table load in the DMA
        # window) — gated only on the ones64 memset.
        nc.scalar.activation(dummy[:], ones64[:, 0:1], FT.Exp, scale=SCALE)

        # PE warm-up spins: the HAM clock gate keeps the PE at 1.2 GHz until
        # it has been busy for a ~3.4us window, and re-throttles after idle
        # windows. Spin garbage matmuls before and *through* the DMA-paced
        # and exp-paced head phases so the real work runs at 2.4 GHz.
        spin_sb = const_pool.tile([128, 512], BF16)
        nc.vector.memset(spin_sb[:], 1.0)
        spin_ps = ps_proj.tile([128, 2, 512], F32, tag="proj", name="spin")

        def spin(n):
            for i in range(n):
                nc.tensor.matmul(
                    spin_ps[0:64, i % 2],
                    lhsT=ones64[:],
                    rhs=spin_sb[:],
                    start=True,
                    stop=True,
                )

        spin(12)

        def qkT_proj_unit(xT_sb, wqk_sb, jt, lh):
            """qkT[j, l] = sum_c wqk[c, j] xT[c, l] for one (j-tile, L-half)."""
            ps = ps_proj.tile([128, 2, 512], F32, tag="proj")
            for ct in range(CT):
                for lc in range(2):
                    nc.tensor.matmul(
                        ps[:, lc],
                        lhsT=wqk_sb[:, ct, jt * 128 : (jt + 1) * 128],
                        rhs=xT_sb[:, ct, lh * 1024 + lc * 512 : lh * 1024 + (lc + 1) * 512],
                        start=(ct == 0),
                        stop=(ct == CT - 1),
                    )
            nc.vector.tensor_scalar_add(
                qkT_sb[:, jt, 2 * lh : 2 * lh + 2, :], ps[:], bqk_sb[:, jt : jt + 1]
            )

        def score_kt(p, jq, expT, kt):
            """One k-tile of scoresT + its exp for head pair p, chunk jq.
            exp runs on ACT (true exp, scale fused) or on DVE (Schraudolph
            fused-multiply-add writing the bf16 bit pattern as int16)."""
            S = ps_s.tile([128, 2, 512], F32, tag="s")
            jl, off = kt // 4, (kt % 4) * 128
            nc.tensor.matmul(
                S[:, 0],
                lhsT=qkT_sb[0:64, 4 + p, jl, off : off + 128],
                rhs=qkT_sb[0:64, p, jq, :],
                start=True,
                stop=True,
            )
            nc.tensor.matmul(
                S[:, 1],
                lhsT=qkT_sb[64:128, 4 + p, jl, off : off + 128],
                rhs=qkT_sb[64:128, p, jq, :],
                start=True,
                stop=True,
            )
            if kt in DVE_KT:
                nc.vector.tensor_scalar(
                    expT[:, kt].bitcast(I16), S[:], SCH_A, SCH_B, MULT, ADD
                )
            else:
                nc.scalar.activation(expT[:, kt], S[:], FT.Exp, scale=SCALE)

        def av_mms(av, p, expT, kts):
            """AV accumulation groups with the row sums fused: per head a
            [65, 512] group whose 65th row (ones column of V65) accumulates
            the softmax denominators. Head A -> av[:, 0], head B -> av[:, 1]."""
            hA, hB = 2 * p, 2 * p + 1
            for kt in kts:
                st, sp = kt == 0, kt == KT - 1
                nc.tensor.matmul(
                    av[0:65, 0],
                    lhsT=V65_sb[:, kt, hA, :],
                    rhs=expT[:, kt, 0],
                    start=st,
                    stop=sp,
                )
                nc.tensor.matmul(
                    av[0:65, 1],
                    lhsT=V65_sb[:, kt, hB, :],
                    rhs=expT[:, kt, 1],
                    start=st,
                    stop=sp,
                )

        def sums_stage(av):
            """Stage the sums row in SBUF (ACT engine, off the DVE path) and
            broadcast it across PSUM rows 64:128 on the PE, so the reciprocal
            runs as full-lane [64, 2, 512] DVE ops like the v1 norm."""
            sums_sb = r_pool.tile([128, 2, 512], BF16, tag="sums")
            nc.scalar.copy(sums_sb[64:65], av[64:65, :, :])
            for hp in range(2):
                nc.tensor.matmul(
                    av[64:128, hp],
                    lhsT=ones64[64:65, :],
                    rhs=sums_sb[64:65, hp],
                    start=True,
                    stop=True,
                    skip_group_check=True,
                )

        def recip_norm_head(av, hp, p, jq):
            """1/sums for one head on the replicated rows (bit-trick seed +
            one Newton step, full-lane [64, 512] DVE ops), then the
            partition-shifted normalize multiply into outT. Scratch tiles are
            shared between heads (the chains serialize on the DVE FIFO)."""
            r0 = r_pool.tile([128, 512], I32, tag="r0", name=f"r0_{p}_{jq}_{hp}")
            nc.vector.tensor_scalar(
                r0[64:128], av[64:128, hp].bitcast(I32), -1.0, RCP_MAGIC, MULT, ADD
            )
            t_sb = r_pool.tile([128, 512], F32, tag="t", name=f"t_{p}_{jq}_{hp}")
            nc.vector.tensor_tensor(
                t_sb[64:128], av[64:128, hp], r0[64:128].bitcast(F32), MULT
            )
            nc.vector.tensor_scalar(t_sb[64:128], t_sb[64:128], -1.0, 2.0, MULT, ADD)
            r1 = r_pool.tile([128, 512], F32, tag="r1", name=f"r1_{p}_{jq}_{hp}")
            nc.vector.tensor_tensor(
                r1[64:128], t_sb[64:128], r0[64:128].bitcast(F32), MULT
            )
            cols = slice(jq * 512, (jq + 1) * 512)
            rows = slice(64 * hp, 64 * hp + 64)
            nc.vector.tensor_tensor(
                outT_sb[rows, p, cols], av[0:64, hp], r1[64:128], MULT
            )

        def y_unit_slices(lt, pool=None, tag="proj"):
            """y[l, f] = sum_d outT[d, l] wo[d, f] for one l-tile, split into
            two drippable half-slices (one per 512-wide f chunk), each with
            its own SBUF staging half + output DMA. Copies alternate DVE and
            ACT; DMAs alternate the gpsimd/sync queues. The last chunk routes
            psy through the (by then idle) score banks so consecutive l-tile
            groups pipeline instead of waiting on each other's copies."""
            box = {}
            pool = ps_proj if pool is None else pool

            def emit(fc, lt=lt):
                if fc == 0:
                    box["psy"] = pool.tile(
                        [128, 2, 512], F32, tag=tag, name=f"psy_{lt}"
                    )
                psy = box["psy"]
                y_sb = y_pool.tile([128, 512], BF16, tag="y", name=f"y_{lt}_{fc}")
                for p in range(NP):
                    nc.tensor.matmul(
                        psy[:, fc],
                        lhsT=outT_sb[:, p, lt * 128 : (lt + 1) * 128],
                        rhs=wo_sb[:, p, fc * 512 : (fc + 1) * 512],
                        start=(p == 0),
                        stop=(p == NP - 1),
                    )
                if fc == 0:
                    nc.vector.tensor_copy(y_sb[:], psy[:, fc])
                else:
                    nc.scalar.copy(y_sb[:], psy[:, fc])
                deng = nc.gpsimd if (lt + fc) % 2 == 0 else nc.sync
                deng.dma_start(
                    y_d[lt * 128 : (lt + 1) * 128, fc * 512 : (fc + 1) * 512],
                    y_sb[:],
                )

            return [lambda fc=fc: emit(fc) for fc in range(2)]

        with tc.tile_pool(name="xw", bufs=1) as xw_pool:
            # ct-interleaved input DMAs: the first projection units consume
            # c-tiles in order, so they start as soon as tile 0 lands.
            xT_sb = xw_pool.tile([128, CT, L], BF16)
            wqk_sb = xw_pool.tile([128, CT, 1024], BF16)
            xT_r = xT_d.rearrange("(t p) l -> p t l", p=128)
            wqk_r = wqk_d.rearrange("(t p) j -> p t j", p=128)
            # wqk rides the scalar queue so xT (the fatter stream) has the
            # sync queue to itself — inputs land ~5us sooner
            for ct in range(CT):
                nc.scalar.dma_start(wqk_sb[:, ct], wqk_r[:, ct])
                nc.sync.dma_start(xT_sb[:, ct], xT_r[:, ct])

            vbox = {}

            def V_proj_unit(wv_sb, lt):
                if lt % 2 == 0:
                    vbox["psv"] = ps_av.tile(
                        [128, 2, 512], F32, tag="av", name=f"psv_{lt}"
                    )
                psv = vbox["psv"]
                for ct in range(CT):
                    nc.tensor.matmul(
                        psv[:, lt % 2],
                        lhsT=xT_sb[:, ct, lt * 128 : (lt + 1) * 128],
                        rhs=wv_sb[:, ct, :],
                        start=(ct == 0),
                        stop=(ct == CT - 1),
                    )
                # strided copy: psum cols h*64+d -> V65[:, lt, h, d]
                nc.vector.tensor_copy(V65_sb[:, lt, :, 0:64], psv[:, lt % 2])

            def qkT_unit_slices(jt, lh):
                """A qkT projection unit split into 4 drippable slices of
                4 matmuls (the psum accumulation group spans the slices)."""
                box = {}

                def emit(i, jt=jt, lh=lh):
                    if i == 0:
                        box["ps"] = ps_proj.tile(
                            [128, 2, 512], F32, tag="proj", name=f"proj_{jt}_{lh}"
                        )
                    ps = box["ps"]
                    for ct in (2 * i, 2 * i + 1):
                        for lc in range(2):
                            nc.tensor.matmul(
                                ps[:, lc],
                                lhsT=wqk_sb[:, ct, jt * 128 : (jt + 1) * 128],
                                rhs=xT_sb[
                                    :,
                                    ct,
                                    lh * 1024 + lc * 512 : lh * 1024 + (lc + 1) * 512,
                                ],
                                start=(ct == 0),
                                stop=(ct == CT - 1),
                            )
                    if i == 3:
                        nc.vector.tensor_scalar_add(
                            qkT_sb[:, jt, 2 * lh : 2 * lh + 2, :],
                            ps[:],
                            bqk_sb[:, jt : jt + 1],
                        )

                return [lambda i=i: emit(i) for i in range(4)]

            # Emission schedule: per chunk c we emit its AV groups (paced by
            # its exps), then the first 4 score k-tiles of chunk c+1 woven
            # between the two halves of c's row-sum pass (the sums can only
            # start once the AV groups close, i.e. after c's last exp), then
            # the normalize, then the remaining score k-tiles of c+1 with
            # projection work dripped one slice per k-tile.
            with tc.tile_pool(name="wv", bufs=1) as wv_pool:
                wv_sb = wv_pool.tile([128, CT, 512], BF16)
                nc.scalar.dma_start(wv_sb[:], wv_d.rearrange("(t p) j -> p t j", p=128))

                # pair 0 q/k projections up front, interleaved per c-tile so
                # the PE stays dense while the input DMA streams. They ride
                # the (otherwise still idle) score PSUM banks so both can be
                # in flight at once.
                chunks = [(p, jq) for p in range(NP) for jq in range(JQ)]
                psA = ps_s.tile([128, 2, 512], F32, tag="s", name="projA")
                psB = ps_s.tile([128, 2, 512], F32, tag="s", name="projB")
                for ct in range(CT):
                    spin(2)
                    for ps, jt in ((psA, 4), (psB, 0)):
                        for lc in range(2):
                            nc.tensor.matmul(
                                ps[:, lc],
                                lhsT=wqk_sb[:, ct, jt * 128 : (jt + 1) * 128],
                                rhs=xT_sb[:, ct, lc * 512 : (lc + 1) * 512],
                                start=(ct == 0),
                                stop=(ct == CT - 1),
                            )
                for ps, jt in ((psA, 4), (psB, 0)):
                    nc.vector.tensor_scalar_add(
                        qkT_sb[:, jt, 0:2, :], ps[:], bqk_sb[:, jt : jt + 1]
                    )
                exp0 = exp_pool.tile([128, KT, 2, 512], BF16, tag="expT")
                for kt in range(8):
                    score_kt(0, 0, exp0, kt)
                    spin(2)
                qkT_proj_unit(xT_sb, wqk_sb, 4, 1)
                for kt in range(8, KT):
                    score_kt(0, 0, exp0, kt)
                    spin(1)
                # chunk (0,1) scores with the V projection dripped per k-tile
                exp1 = exp_pool.tile([128, KT, 2, 512], BF16, tag="expT")
                for kt in range(KT):
                    score_kt(0, 1, exp1, kt)
                    V_proj_unit(wv_sb, kt)
                qkT_proj_unit(xT_sb, wqk_sb, 0, 1)

            exps = {0: exp0, 1: exp1}
            for ci in range(len(chunks)):
                p, jq = chunks[ci]
                nxt = chunks[ci + 1] if ci + 1 < len(chunks) else None
                emit_nxt = nxt is not None and (ci + 1) not in exps
                if emit_nxt:
                    exps[ci + 1] = exp_pool.tile([128, KT, 2, 512], BF16, tag="expT", name=f"expT_{ci+1}")
                expT = exps.pop(ci)
                av = ps_av.tile([128, 2, 512], F32, tag="av", name=f"av_{ci}")
                av_mms(av, p, expT, range(0, KT - 1))
                if emit_nxt:
                    # runs during this chunk's last exp (S slot frees at kt14)
                    score_kt(*nxt, exps[ci + 1], 0)
                av_mms(av, p, expT, [KT - 1])
                sums_stage(av)
                if emit_nxt:
                    score_kt(*nxt, exps[ci + 1], 1)
                    score_kt(*nxt, exps[ci + 1], 2)
                recip_norm_head(av, 0, p, jq)
                if emit_nxt:
                    score_kt(*nxt, exps[ci + 1], 3)
                recip_norm_head(av, 1, p, jq)
                if emit_nxt:
                    score_kt(*nxt, exps[ci + 1], 4)

                # filler: next pair's projections (pairs 0-2) or the
                # output projection (pair 3), dripped per score k-tile
                if p < NP - 1:
                    nj = p + 1
                    jt, lh = [(4 + nj, 0), (4 + nj, 1), (nj, 0), (nj, 1)][jq]
                    drip = qkT_unit_slices(jt, lh)
                else:
                    drip = []
                    last = ci == len(chunks) - 1
                    for lt in range(4 * jq, 4 * jq + 4):
                        if last and lt < 4 * jq + 2:
                            drip.extend(y_unit_slices(lt, pool=ps_s, tag="s"))
                        else:
                            drip.extend(y_unit_slices(lt))
                if nxt is None:
                    # final chunk: no next-chunk scores to fill the PE while
                    # the norm chain runs on DVE — spin so the HAM clock gate
                    # stays released for the closing y-projection burst
                    spin2 = ps_proj.tile([128, 2, 512], F32, tag="proj", name="spin2")
                    for i in range(14):
                        nc.tensor.matmul(
                            spin2[0:64, i % 2],
                            lhsT=ones64[:],
                            rhs=spin_sb[:],
                            start=True,
                            stop=True,
                        )
                for kt in range(5, KT):
                    if emit_nxt:
                        score_kt(*nxt, exps[ci + 1], kt)
                    if drip:
                        drip.pop(0)()
                while drip:
                    drip.pop(0)()

    nc.finalize()
    return nc


_NC_CACHE = None


def _get_program():
    global _NC_CACHE
    if _NC_CACHE is None:
        _NC_CACHE = _build_program()
    return _NC_CACHE


def _make_in_maps(x, W_in, b_in, W_out):
    bf = ml_dtypes.bfloat16
    in_maps = []
    for c in range(NCORES):
        n, g = c // 2, c % 2
        h0 = g * HG  # first global head
        j0 = h0 * D  # 512*g
        xT = np.ascontiguousarray(x[n].T).astype(bf)  # [C, L]
        wqk = np.concatenate(
            [W_in[:, j0 : j0 + 512], W_in[:, QKV + j0 : QKV + j0 + 512]], axis=1
        ).astype(bf)
        wv = np.ascontiguousarray(W_in[:, 2 * QKV + j0 : 2 * QKV + j0 + 512]).astype(bf)
        wo = np.ascontiguousarray(W_out[j0 : j0 + 512, :]).astype(bf)
        bqk = (
            np.concatenate([b_in[j0 : j0 + 512], b_in[QKV + j0 : QKV + j0 + 512]])
            .astype(np.float32)
            .reshape(8, 128)
            .T.copy()
        )
        in_maps.append({"xT": xT, "wqk": wqk, "wv": wv, "wo": wo, "bqk": bqk})
    return in_maps


def kernel(x, W_in, b_in, W_out, b_out):
    global LAST_RESULTS
    x = np.asarray(x, dtype=np.float32)
    W_in = np.asarray(W_in, dtype=np.float32)
    b_in = np.asarray(b_in, dtype=np.float32)
    W_out = np.asarray(W_out, dtype=np.float32)
    b_out = np.asarray(b_out, dtype=np.float32)

    nc = _get_program()
    in_maps = _make_in_maps(x, W_in, b_in, W_out)
    res = run_bass_kernel_spmd(nc, in_maps, list(range(NCORES)), trace=TRACE)
    LAST_RESULTS = res

    # host bias: b_out + b_v @ W_out  (b_v enters linearly through the
    # softmax-normalized value average: A@(V+b_v) = A@V + b_v)
    host_bias = (
        b_out.astype(np.float64)
        + b_in[2 * QKV :].astype(np.float64) @ W_out.astype(np.float64)
    ).astype(np.float32)

    out = np.empty((N, L, F), dtype=np.float32)
    for n in range(N):
        y0 = np.asarray(res.results[2 * n]["y"], dtype=np.float32)
        y1 = np.asarray(res.results[2 * n + 1]["y"], dtype=np.float32)
        out[n] = y0 + y1 + host_bias
    return out


# revision 20
# speedup vs baseline: 1.2137x; 1.2137x over previous
"""Multi-head attention (N=4, L=2048, C=1024, H=16, D=64) on 8 TRN2 NeuronCores.

Sharding: core c -> batch n = c//2, head-group g = c%2 (8 heads each).
Each core computes its 8 heads' attention + the partial output projection
for batch n; the host sums the two partials per batch and adds the
constant bias term (b_out + b_v @ W_out).

Device-side layout (per core):
  xT   [C=1024, L=2048]  bf16 (x[n].T, host-transposed/cast)
  wqk  [C, 1024]         bf16 (W_in cols: 8 heads' q dims then k dims)
  wv   [C, 512]          bf16 (W_in cols: 8 heads' v dims)
  wo   [512, F=1024]     bf16 (W_out rows for the 8 heads)
  bqk  [128, 8]          f32  (q/k bias, partition-major per j-tile)
  y    [L, F]            f32  output partial

Pipeline (exp split across ACT+DVE paces the chunks; PE hides under it):
  - PE warm-up spin matmuls + exp-table preload run during the input DMA
    window so the HAM clock gate is released before real work starts; the
    first two q/k projection units are interleaved per c-tile (through the
    score PSUM banks) to keep the PE dense while the DMA streams
  - qT/kT = W^T @ xT (j on partitions), V = xT^T @ Wv (l on partitions)
  - scoresT[k, q] per head, row-tiled head pairs (K=64 -> rows 0-63 / 64-127)
  - exp: 11/16 k-tiles on ACT (scale=1/8 fused, fp32 PSUM -> bf16 SBUF),
    5/16 on DVE via Schraudolph (one fused multiply-add emitting the bf16
    bit pattern of 2^t as int16) so the two exp streams run concurrently
  - AV^T as col-tiled pairs into avD (head A rows 0:64, head B 64:128);
    row sums as M=64 ones-matmuls into avS with the same partition split,
    so 1/sum (bit-trick seed + one Newton step, whole-tile DVE ops) and
    the normalize multiply are single [128,512] instructions per chunk
  - qkT projections for the next pair and the final y projection are
    interleaved into the attention chunks as PE filler work; y copies
    alternate DVE/ACT and y DMAs alternate the gpsimd/sync queues
"""

import sys
from contextlib import ExitStack

import numpy as np

sys.path.insert(0, "/opt/trn_rl_repo")

import ml_dtypes

import concourse.bass as bass
import concourse.tile as tile
from concourse import bacc, mybir
from concourse.bass_utils import run_bass_kernel_spmd

BF16 = mybir.dt.bfloat16
F32 = mybir.dt.float32
I16 = mybir.dt.int16
I32 = mybir.dt.int32
FT = mybir.ActivationFunctionType
MULT = mybir.AluOpType.mult
ADD = mybir.AluOpType.add

N, L, C, H, D = 4, 2048, 1024, 16, 64
QKV = H * D  # 1024
F = 1024  # output feature dim
HG = 8  # heads per core
NCORES = 8
SCALE = float(D) ** -0.5  # 0.125

CT = C // 128  # 8 c-tiles
LT = L // 128  # 16 l-tiles
JQ = L // 512  # 4 q-chunks
KT = L // 128  # 16 k-tiles
NP = HG // 2  # 4 head pairs

# Softmax-exp engine split: k-tiles in DVE_KT evaluate exp on the vector
# engine via the Schraudolph bit trick (bf16 bit pattern of 2^t built with
# one fused multiply-add, written as int16), the rest on ACT. Spreading the
# DVE tiles through the chunk keeps both exp streams concurrent.
DVE_KT = frozenset({2, 5, 8, 11, 14})
SCH_A = SCALE * 128.0 / float(np.log(2.0))  # score -> bf16-bit scale
SCH_B = 16250.5  # centered exponent-bias constant
RCP_MAGIC = float(0x7EF30000)  # Newton seed: r0_bits = MAGIC - x_bits

# Globals for test harness introspection
TRACE = False
LAST_RESULTS = None


def _build_program() -> bass.Bass:
    nc = bacc.Bacc()

    xT_d = nc.declare_dram_parameter("xT", [C, L], BF16, isOutput=False)
    wqk_d = nc.declare_dram_parameter("wqk", [C, 1024], BF16, isOutput=False)
    wv_d = nc.declare_dram_parameter("wv", [C, 512], BF16, isOutput=False)
    wo_d = nc.declare_dram_parameter("wo", [512, F], BF16, isOutput=False)
    bqk_d = nc.declare_dram_parameter("bqk", [128, 8], F32, isOutput=False)
    y_d = nc.declare_dram_parameter("y", [L, F], BF16, isOutput=True)

    with tile.TileContext(nc) as tc, ExitStack() as ctx:
        const_pool = ctx.enter_context(tc.tile_pool(name="const", bufs=1))
        qk_pool = ctx.enter_context(tc.tile_pool(name="qkT", bufs=1))
        v_pool = ctx.enter_context(tc.tile_pool(name="V", bufs=1))
        outT_pool = ctx.enter_context(tc.tile_pool(name="outT", bufs=1))
        exp_pool = ctx.enter_context(tc.tile_pool(name="expT", bufs=2))
        r_pool = ctx.enter_context(tc.tile_pool(name="r", bufs=1))
        y_pool = ctx.enter_context(tc.tile_pool(name="y", bufs=2))
        wo_pool = ctx.enter_context(tc.tile_pool(name="wo", bufs=1))
        # PSUM: scores 2x2 banks + avD 1 + avS 1 + proj 2 = 8 banks
        ps_s = ctx.enter_context(tc.tile_pool(name="ps_s", bufs=2, space="PSUM"))
        ps_avD = ctx.enter_context(tc.tile_pool(name="ps_avD", bufs=1, space="PSUM"))
        ps_avS = ctx.enter_context(tc.tile_pool(name="ps_avS", bufs=1, space="PSUM"))
        ps_proj = ctx.enter_context(tc.tile_pool(name="ps_proj", bufs=1, space="PSUM"))

        ones64 = const_pool.tile([128, 64], BF16)
        nc.vector.memset(ones64[:], 1.0)
        dummy = const_pool.tile([128, 1], F32)
        bqk_sb = const_pool.tile([128, 8], F32)
        nc.sync.dma_start(bqk_sb[:], bqk_d[:])
        wo_sb = wo_pool.tile([128, 4, F], BF16)

        # qT/kT: [128, jt(8), jl(4), 512] ; jt 0-3 q dims, 4-7 k dims.
        qkT_sb = qk_pool.tile([128, 8, 4, 512], BF16)
        # V: [128, lt(16), 512]
        V_sb = v_pool.tile([128, LT, 512], BF16)
        # outT: [128, pair(4), L] (partitions = 2 heads x 64 dims)
        outT_sb = outT_pool.tile([128, NP, L], BF16)

        # Exp-table preload on ACT (hides the ~1.3us table load in the DMA
        # window) — gated only on the ones64 memset.
        nc.scalar.activation(dummy[:], ones64[:, 0:1], FT.Exp, scale=SCALE)

        # PE warm-up spins: the HAM clock gate keeps the PE at 1.2 GHz until
        # it has been busy for a ~3.4us window, and re-throttles after idle
        # windows. Spin garbage matmuls before and *through* the DMA-paced
        # and exp-paced head phases so the real work runs at 2.4 GHz.
        spin_sb = const_pool.tile([128, 512], BF16)
        nc.vector.memset(spin_sb[:], 1.0)
        spin_ps = ps_proj.tile([128, 2, 512], F32, tag="proj", name="spin")

        def spin(n):
            for i in range(n):
                nc.tensor.matmul(
                    spin_ps[0:64, i % 2],
                    lhsT=ones64[:],
                    rhs=spin_sb[:],
                    start=True,
                    stop=True,
                )

        spin(12)

        def qkT_proj_unit(xT_sb, wqk_sb, jt, lh):
            """qkT[j, l] = sum_c wqk[c, j] xT[c, l] for one (j-tile, L-half)."""
            ps = ps_proj.tile([128, 2, 512], F32, tag="proj")
            for ct in range(CT):
                for lc in range(2):
                    nc.tensor.matmul(
                        ps[:, lc],
                        lhsT=wqk_sb[:, ct, jt * 128 : (jt + 1) * 128],
                        rhs=xT_sb[:, ct, lh * 1024 + lc * 512 : lh * 1024 + (lc + 1) * 512],
                        start=(ct == 0),
                        stop=(ct == CT - 1),
                    )
            nc.vector.tensor_scalar_add(
                qkT_sb[:, jt, 2 * lh : 2 * lh + 2, :], ps[:], bqk_sb[:, jt : jt + 1]
            )

        def score_kt(p, jq, expT, kt):
            """One k-tile of scoresT + its exp for head pair p, chunk jq.
            exp runs on ACT (true exp, scale fused) or on DVE (Schraudolph
            fused-multiply-add writing the bf16 bit pattern as int16)."""
            S = ps_s.tile([128, 2, 512], F32, tag="s")
            jl, off = kt // 4, (kt % 4) * 128
            nc.tensor.matmul(
                S[:, 0],
                lhsT=qkT_sb[0:64, 4 + p, jl, off : off + 128],
                rhs=qkT_sb[0:64, p, jq, :],
                start=True,
                stop=True,
            )
            nc.tensor.matmul(
                S[:, 1],
                lhsT=qkT_sb[64:128, 4 + p, jl, off : off + 128],
                rhs=qkT_sb[64:128, p, jq, :],
                start=True,
                stop=True,
            )
            if kt in DVE_KT:
                nc.vector.tensor_scalar(
                    expT[:, kt].bitcast(I16), S[:], SCH_A, SCH_B, MULT, ADD
                )
            else:
                nc.scalar.activation(expT[:, kt], S[:], FT.Exp, scale=SCALE)

        def av_alloc():
            avD = ps_avD.tile([128, 512], F32, tag="avD")
            avS = ps_avS.tile([128, 512], F32, tag="avS")
            return avD, avS

        def av_mms(avD, avS, p, jq, expT, kts):
            """AV accumulation-group matmuls: head A -> avD rows 0:64,
            head B -> avD rows 64:128 (concurrent col-tile pair)."""
            hA, hB = 2 * p, 2 * p + 1
            for kt in kts:
                st, sp = kt == 0, kt == KT - 1
                nc.tensor.matmul(
                    avD[0:64],
                    lhsT=V_sb[:, kt, hA * 64 : hA * 64 + 64],
                    rhs=expT[:, kt, 0],
                    start=st,
                    stop=sp,
                )
                nc.tensor.matmul(
                    avD[64:128],
                    lhsT=V_sb[:, kt, hB * 64 : hB * 64 + 64],
                    rhs=expT[:, kt, 1],
                    start=st,
                    stop=sp,
                )

        def sum_mms(avD, avS, expT, kts):
            """Row sums, replicated across 64 partitions (M=64 ones): head A
            -> avS rows 0:64, head B -> avS rows 64:128, partition-aligned
            with avD so the reciprocal/normalize are single whole-tile ops."""
            for kt in kts:
                st, sp = kt == 0, kt == KT - 1
                nc.tensor.matmul(
                    avS[0:64], lhsT=ones64[:], rhs=expT[:, kt, 0], start=st, stop=sp
                )
                nc.tensor.matmul(
                    avS[64:128], lhsT=ones64[:], rhs=expT[:, kt, 1], start=st, stop=sp
                )

        def norm_part(p, jq, avD, avS):
            # 1/sums via bit-trick seed + one Newton step (all [128,512], both
            # heads at once), then a single PSUM-side multiply into outT.
            r0 = r_pool.tile([128, 512], I32, tag="r0")
            nc.vector.tensor_scalar(
                r0[:], avS[:].bitcast(I32), -1.0, RCP_MAGIC, MULT, ADD
            )
            t_sb = r_pool.tile([128, 512], F32, tag="t")
            nc.vector.tensor_tensor(t_sb[:], avS[:], r0[:].bitcast(F32), MULT)
            u_sb = r_pool.tile([128, 512], F32, tag="u")
            nc.vector.tensor_scalar(u_sb[:], t_sb[:], -1.0, 2.0, MULT, ADD)
            r1 = r_pool.tile([128, 512], F32, tag="r1")
            nc.vector.tensor_tensor(r1[:], u_sb[:], r0[:].bitcast(F32), MULT)
            cols = slice(jq * 512, (jq + 1) * 512)
            nc.vector.tensor_tensor(outT_sb[:, p, cols], avD[:], r1[:], MULT)

        def y_unit_slices(lt, pool=None, tag="proj"):
            """y[l, f] = sum_d outT[d, l] wo[d, f] for one l-tile, split into
            two drippable half-slices (one per 512-wide f chunk), each with
            its own SBUF staging half + output DMA. Copies alternate DVE and
            ACT; DMAs alternate the gpsimd/sync queues. The last chunk routes
            psy through the (by then idle) score banks so consecutive l-tile
            groups pipeline instead of waiting on each other's copies."""
            box = {}
            pool = ps_proj if pool is None else pool

            def emit(fc, lt=lt):
                if fc == 0:
                    box["psy"] = pool.tile(
                        [128, 2, 512], F32, tag=tag, name=f"psy_{lt}"
                    )
                psy = box["psy"]
                y_sb = y_pool.tile([128, 512], BF16, tag="y", name=f"y_{lt}_{fc}")
                for p in range(NP):
                    nc.tensor.matmul(
                        psy[:, fc],
                        lhsT=outT_sb[:, p, lt * 128 : (lt + 1) * 128],
                        rhs=wo_sb[:, p, fc * 512 : (fc + 1) * 512],
                        start=(p == 0),
                        stop=(p == NP - 1),
                    )
                if fc == 0:
                    nc.vector.tensor_copy(y_sb[:], psy[:, fc])
                else:
                    nc.scalar.copy(y_sb[:], psy[:, fc])
                deng = nc.gpsimd if (lt + fc) % 2 == 0 else nc.sync
                deng.dma_start(
                    y_d[lt * 128 : (lt + 1) * 128, fc * 512 : (fc + 1) * 512],
                    y_sb[:],
                )

            return [lambda fc=fc: emit(fc) for fc in range(2)]

        with tc.tile_pool(name="xw", bufs=1) as xw_pool:
            # ct-interleaved input DMAs: the first projection units consume
            # c-tiles in order, so they start as soon as tile 0 lands.
            xT_sb = xw_pool.tile([128, CT, L], BF16)
            wqk_sb = xw_pool.tile([128, CT, 1024], BF16)
            xT_r = xT_d.rearrange("(t p) l -> p t l", p=128)
            wqk_r = wqk_d.rearrange("(t p) j -> p t j", p=128)
            # wqk rides the scalar queue; xT alternates the sync/gpsimd
            # queues so the 4MB stream lands in about half the time. wo
            # follows on gpsimd (its first consumer is ~300us in).
            for ct in range(CT):
                nc.scalar.dma_start(wqk_sb[:, ct], wqk_r[:, ct])
                xeng = nc.sync if ct % 2 == 0 else nc.gpsimd
                xeng.dma_start(xT_sb[:, ct], xT_r[:, ct])
            nc.gpsimd.dma_start(wo_sb[:], wo_d.rearrange("(t p) f -> p t f", p=128))

            def V_proj_unit(wv_sb, lt):
                pool, tag = (ps_avD, "avD") if lt % 2 == 0 else (ps_avS, "avS")
                psv = pool.tile([128, 512], F32, tag=tag)
                for ct in range(CT):
                    nc.tensor.matmul(
                        psv[:],
                        lhsT=xT_sb[:, ct, lt * 128 : (lt + 1) * 128],
                        rhs=wv_sb[:, ct, :],
                        start=(ct == 0),
                        stop=(ct == CT - 1),
                    )
                nc.vector.tensor_copy(V_sb[:, lt, :], psv[:])

            def qkT_unit_slices(jt, lh):
                """A qkT projection unit split into 4 drippable slices of
                4 matmuls (the psum accumulation group spans the slices)."""
                box = {}

                def emit(i, jt=jt, lh=lh):
                    if i == 0:
                        box["ps"] = ps_proj.tile(
                            [128, 2, 512], F32, tag="proj", name=f"proj_{jt}_{lh}"
                        )
                    ps = box["ps"]
                    for ct in (2 * i, 2 * i + 1):
                        for lc in range(2):
                            nc.tensor.matmul(
                                ps[:, lc],
                                lhsT=wqk_sb[:, ct, jt * 128 : (jt + 1) * 128],
                                rhs=xT_sb[
                                    :,
                                    ct,
                                    lh * 1024 + lc * 512 : lh * 1024 + (lc + 1) * 512,
                                ],
                                start=(ct == 0),
                                stop=(ct == CT - 1),
                            )
                    if i == 3:
                        nc.vector.tensor_scalar_add(
                            qkT_sb[:, jt, 2 * lh : 2 * lh + 2, :],
                            ps[:],
                            bqk_sb[:, jt : jt + 1],
                        )

                return [lambda i=i: emit(i) for i in range(4)]

            # Emission schedule: per chunk c we emit its AV groups (paced by
            # its exps), then the first 4 score k-tiles of chunk c+1 woven
            # between the two halves of c's row-sum pass (the sums can only
            # start once the AV groups close, i.e. after c's last exp), then
            # the normalize, then the remaining score k-tiles of c+1 with
            # projection work dripped one slice per k-tile.
            with tc.tile_pool(name="wv", bufs=1) as wv_pool:
                wv_sb = wv_pool.tile([128, CT, 512], BF16)
                nc.scalar.dma_start(wv_sb[:], wv_d.rearrange("(t p) j -> p t j", p=128))

                # pair 0 q/k projections up front, interleaved per c-tile so
                # the PE stays dense while the input DMA streams. They ride
                # the (otherwise still idle) score PSUM banks so both can be
                # in flight at once.
                chunks = [(p, jq) for p in range(NP) for jq in range(JQ)]
                psA = ps_s.tile([128, 2, 512], F32, tag="s", name="projA")
                psB = ps_s.tile([128, 2, 512], F32, tag="s", name="projB")
                for ct in range(CT):
                    spin(2)
                    for ps, jt in ((psA, 4), (psB, 0)):
                        for lc in range(2):
                            nc.tensor.matmul(
                                ps[:, lc],
                                lhsT=wqk_sb[:, ct, jt * 128 : (jt + 1) * 128],
                                rhs=xT_sb[:, ct, lc * 512 : (lc + 1) * 512],
                                start=(ct == 0),
                                stop=(ct == CT - 1),
                            )
                for ps, jt in ((psA, 4), (psB, 0)):
                    nc.vector.tensor_scalar_add(
                        qkT_sb[:, jt, 0:2, :], ps[:], bqk_sb[:, jt : jt + 1]
                    )
                exp0 = exp_pool.tile([128, KT, 2, 512], BF16, tag="expT")
                for kt in range(8):
                    score_kt(0, 0, exp0, kt)
                    spin(3)
                qkT_proj_unit(xT_sb, wqk_sb, 4, 1)
                for kt in range(8, KT):
                    score_kt(0, 0, exp0, kt)
                    spin(2)
                # chunk (0,1) scores with the V projection dripped per k-tile
                exp1 = exp_pool.tile([128, KT, 2, 512], BF16, tag="expT")
                for kt in range(KT):
                    score_kt(0, 1, exp1, kt)
                    V_proj_unit(wv_sb, kt)
                qkT_proj_unit(xT_sb, wqk_sb, 0, 1)

            exps = {0: exp0, 1: exp1}
            for ci in range(len(chunks)):
                p, jq = chunks[ci]
                nxt = chunks[ci + 1] if ci + 1 < len(chunks) else None
                emit_nxt = nxt is not None and (ci + 1) not in exps
                if emit_nxt:
                    exps[ci + 1] = exp_pool.tile([128, KT, 2, 512], BF16, tag="expT", name=f"expT_{ci+1}")
                expT = exps.pop(ci)
                avD, avS = av_alloc()
                av_mms(avD, avS, p, jq, expT, range(0, KT - 1))
                if emit_nxt:
                    # runs during this chunk's last exp (S slot frees at kt14)
                    score_kt(*nxt, exps[ci + 1], 0)
                av_mms(avD, avS, p, jq, expT, [KT - 1])
                if emit_nxt:
                    score_kt(*nxt, exps[ci + 1], 1)
                sum_mms(avD, avS, expT, range(0, 6))
                if emit_nxt:
                    score_kt(*nxt, exps[ci + 1], 2)
                sum_mms(avD, avS, expT, range(6, 11))
                if emit_nxt:
                    score_kt(*nxt, exps[ci + 1], 3)
                sum_mms(avD, avS, expT, range(11, KT))
                if emit_nxt:
                    score_kt(*nxt, exps[ci + 1], 4)
                norm_part(p, jq, avD, avS)

                # filler: next pair's projections (pairs 0-2) or the
                # output projection (pair 3), dripped per score k-tile
                if p < NP - 1:
                    nj = p + 1
                    jt, lh = [(4 + nj, 0), (4 + nj, 1), (nj, 0), (nj, 1)][jq]
                    drip = qkT_unit_slices(jt, lh)
                else:
                    drip = []
                    last = ci == len(chunks) - 1
                    for lt in range(4 * jq, 4 * jq + 4):
                        if last and lt < 4 * jq + 2:
                            drip.extend(y_unit_slices(lt, pool=ps_s, tag="s"))
                        else:
                            drip.extend(y_unit_slices(lt))
                if nxt is None:
                    # final chunk: no next-chunk scores to fill the PE while
                    # the norm chain runs on DVE — spin so the HAM clock gate
                    # stays released for the closing y-projection burst
                    spin2 = ps_proj.tile([128, 2, 512], F32, tag="proj", name="spin2")
                    for i in range(14):
                        nc.tensor.matmul(
                            spin2[0:64, i % 2],
                            lhsT=ones64[:],
                            rhs=spin_sb[:],
                            start=True,
                            stop=True,
                        )
                for kt in range(5, KT):
                    if emit_nxt:
                        score_kt(*nxt, exps[ci + 1], kt)
                    if drip:
                        drip.pop(0)()
                while drip:
                    drip.pop(0)()

    nc.finalize()
    return nc


_NC_CACHE = None


def _get_program():
    global _NC_CACHE
    if _NC_CACHE is None:
        _NC_CACHE = _build_program()
    return _NC_CACHE


def _make_in_maps(x, W_in, b_in, W_out):
    bf = ml_dtypes.bfloat16
    in_maps = []
    for c in range(NCORES):
        n, g = c // 2, c % 2
        h0 = g * HG  # first global head
        j0 = h0 * D  # 512*g
        xT = np.ascontiguousarray(x[n].T).astype(bf)  # [C, L]
        wqk = np.concatenate(
            [W_in[:, j0 : j0 + 512], W_in[:, QKV + j0 : QKV + j0 + 512]], axis=1
        ).astype(bf)
        wv = np.ascontiguousarray(W_in[:, 2 * QKV + j0 : 2 * QKV + j0 + 512]).astype(bf)
        wo = np.ascontiguousarray(W_out[j0 : j0 + 512, :]).astype(bf)
        bqk = (
            np.concatenate([b_in[j0 : j0 + 512], b_in[QKV + j0 : QKV + j0 + 512]])
            .astype(np.float32)
            .reshape(8, 128)
            .T.copy()
        )
        in_maps.append({"xT": xT, "wqk": wqk, "wv": wv, "wo": wo, "bqk": bqk})
    return in_maps


def kernel(x, W_in, b_in, W_out, b_out):
    global LAST_RESULTS
    x = np.asarray(x, dtype=np.float32)
    W_in = np.asarray(W_in, dtype=np.float32)
    b_in = np.asarray(b_in, dtype=np.float32)
    W_out = np.asarray(W_out, dtype=np.float32)
    b_out = np.asarray(b_out, dtype=np.float32)

    nc = _get_program()
    in_maps = _make_in_maps(x, W_in, b_in, W_out)
    res = run_bass_kernel_spmd(nc, in_maps, list(range(NCORES)), trace=TRACE)
    LAST_RESULTS = res

    # host bias: b_out + b_v @ W_out  (b_v enters linearly through the
    # softmax-normalized value average: A@(V+b_v) = A@V + b_v)
    host_bias = (
        b_out.astype(np.float64)
        + b_in[2 * QKV :].astype(np.float64) @ W_out.astype(np.float64)
    ).astype(np.float32)

    out = np.empty((N, L, F), dtype=np.float32)
    for n in range(N):
        y0 = np.asarray(res.results[2 * n]["y"], dtype=np.float32)
        y1 = np.asarray(res.results[2 * n + 1]["y"], dtype=np.float32)
        out[n] = y0 + y1 + host_bias
    return out


# revision 21
# speedup vs baseline: 1.2210x; 1.0061x over previous
"""Multi-head attention (N=4, L=2048, C=1024, H=16, D=64) on 8 TRN2 NeuronCores.

Sharding: core c -> batch n = c//2, head-group g = c%2 (8 heads each).
Each core computes its 8 heads' attention + the partial output projection
for batch n; the host sums the two partials per batch and adds the
constant bias term (b_out + b_v @ W_out).

Device-side layout (per core):
  xT   [C=1024, L=2048]  bf16 (x[n].T, host-transposed/cast)
  wqk  [C, 1024]         bf16 (W_in cols: 8 heads' q dims then k dims)
  wv   [C, 512]          bf16 (W_in cols: 8 heads' v dims)
  wo   [512, F=1024]     bf16 (W_out rows for the 8 heads)
  bqk  [128, 8]          f32  (q/k bias, partition-major per j-tile)
  y    [L, F]            f32  output partial

Pipeline (exp split across ACT+DVE paces the chunks; PE hides under it):
  - PE warm-up spin matmuls + exp-table preload run during the input DMA
    window so the HAM clock gate is released before real work starts; the
    first two q/k projection units are interleaved per c-tile (through the
    score PSUM banks) to keep the PE dense while the DMA streams
  - qT/kT = W^T @ xT (j on partitions), V = xT^T @ Wv (l on partitions)
  - scoresT[k, q] per head, row-tiled head pairs (K=64 -> rows 0-63 / 64-127)
  - exp: 11/16 k-tiles on ACT (scale=1/8 fused, fp32 PSUM -> bf16 SBUF),
    5/16 on DVE via Schraudolph (one fused multiply-add emitting the bf16
    bit pattern of 2^t as int16) so the two exp streams run concurrently
  - AV^T as col-tiled pairs into avD (head A rows 0:64, head B 64:128);
    row sums as M=64 ones-matmuls into avS with the same partition split,
    so 1/sum (bit-trick seed + one Newton step, whole-tile DVE ops) and
    the normalize multiply are single [128,512] instructions per chunk
  - qkT projections for the next pair and the final y projection are
    interleaved into the attention chunks as PE filler work; y copies
    alternate DVE/ACT and y DMAs alternate the gpsimd/sync queues
"""

import sys
from contextlib import ExitStack

import numpy as np

sys.path.insert(0, "/opt/trn_rl_repo")

import ml_dtypes

import concourse.bass as bass
import concourse.tile as tile
from concourse import bacc, mybir
from concourse.bass_utils import run_bass_kernel_spmd

BF16 = mybir.dt.bfloat16
F32 = mybir.dt.float32
I16 = mybir.dt.int16
I32 = mybir.dt.int32
FT = mybir.ActivationFunctionType
MULT = mybir.AluOpType.mult
ADD = mybir.AluOpType.add

N, L, C, H, D = 4, 2048, 1024, 16, 64
QKV = H * D  # 1024
F = 1024  # output feature dim
HG = 8  # heads per core
NCORES = 8
SCALE = float(D) ** -0.5  # 0.125

CT = C // 128  # 8 c-tiles
LT = L // 128  # 16 l-tiles
JQ = L // 512  # 4 q-chunks
KT = L // 128  # 16 k-tiles
NP = HG // 2  # 4 head pairs

# Softmax-exp engine split: k-tiles in DVE_KT evaluate exp on the vector
# engine via the Schraudolph bit trick (bf16 bit pattern of 2^t built with
# one fused multiply-add, written as int16), the rest on ACT. Spreading the
# DVE tiles through the chunk keeps both exp streams concurrent.
DVE_KT = frozenset({2, 5, 8, 11, 14})
SCH_A = SCALE * 128.0 / float(np.log(2.0))  # score -> bf16-bit scale
SCH_B = 16250.5  # centered exponent-bias constant
RCP_MAGIC = float(0x7EF30000)  # Newton seed: r0_bits = MAGIC - x_bits

# Globals for test harness introspection
TRACE = False
LAST_RESULTS = None


def _build_program() -> bass.Bass:
    nc = bacc.Bacc()

    xT_d = nc.declare_dram_parameter("xT", [C, L], BF16, isOutput=False)
    wqk_d = nc.declare_dram_parameter("wqk", [C, 1024], BF16, isOutput=False)
    wv_d = nc.declare_dram_parameter("wv", [C, 512], BF16, isOutput=False)
    wo_d = nc.declare_dram_parameter("wo", [512, F], BF16, isOutput=False)
    bqk_d = nc.declare_dram_parameter("bqk", [128, 8], F32, isOutput=False)
    y_d = nc.declare_dram_parameter("y", [L, F], BF16, isOutput=True)

    with tile.TileContext(nc) as tc, ExitStack() as ctx:
        const_pool = ctx.enter_context(tc.tile_pool(name="const", bufs=1))
        qk_pool = ctx.enter_context(tc.tile_pool(name="qkT", bufs=1))
        v_pool = ctx.enter_context(tc.tile_pool(name="V", bufs=1))
        outT_pool = ctx.enter_context(tc.tile_pool(name="outT", bufs=1))
        exp_pool = ctx.enter_context(tc.tile_pool(name="expT", bufs=2))
        r_pool = ctx.enter_context(tc.tile_pool(name="r", bufs=1))
        y_pool = ctx.enter_context(tc.tile_pool(name="y", bufs=2))
        wo_pool = ctx.enter_context(tc.tile_pool(name="wo", bufs=1))
        # PSUM: scores 2x2 banks + avD 1 + avS 1 + proj 2 = 8 banks
        ps_s = ctx.enter_context(tc.tile_pool(name="ps_s", bufs=2, space="PSUM"))
        ps_avD = ctx.enter_context(tc.tile_pool(name="ps_avD", bufs=1, space="PSUM"))
        ps_avS = ctx.enter_context(tc.tile_pool(name="ps_avS", bufs=1, space="PSUM"))
        ps_proj = ctx.enter_context(tc.tile_pool(name="ps_proj", bufs=1, space="PSUM"))

        ones64 = const_pool.tile([128, 64], BF16)
        nc.vector.memset(ones64[:], 1.0)
        dummy = const_pool.tile([128, 1], F32)
        bqk_sb = const_pool.tile([128, 8], F32)
        nc.sync.dma_start(bqk_sb[:], bqk_d[:])
        wo_sb = wo_pool.tile([128, 4, F], BF16)
        # wo rides the gpsimd DMA queue (idle until the output DMAs start
        # ~300us in): it stops delaying the projection-critical xT/wqk
        # transfers at the head of the sync queue, and it has huge slack
        # (first consumer is the first y unit).
        nc.gpsimd.dma_start(wo_sb[:], wo_d.rearrange("(t p) f -> p t f", p=128))

        # qT/kT: [128, jt(8), jl(4), 512] ; jt 0-3 q dims, 4-7 k dims.
        qkT_sb = qk_pool.tile([128, 8, 4, 512], BF16)
        # V: [128, lt(16), 512]
        V_sb = v_pool.tile([128, LT, 512], BF16)
        # outT: [128, pair(4), L] (partitions = 2 heads x 64 dims)
        outT_sb = outT_pool.tile([128, NP, L], BF16)

        # Exp-table preload on ACT (hides the ~1.3us table load in the DMA
        # window) — gated only on the ones64 memset.
        nc.scalar.activation(dummy[:], ones64[:, 0:1], FT.Exp, scale=SCALE)

        # PE warm-up spins: the HAM clock gate keeps the PE at 1.2 GHz until
        # it has been busy for a ~3.4us window, and re-throttles after idle
        # windows. Spin garbage matmuls before and *through* the DMA-paced
        # and exp-paced head phases so the real work runs at 2.4 GHz.
        spin_sb = const_pool.tile([128, 512], BF16)
        nc.vector.memset(spin_sb[:], 1.0)
        spin_ps = ps_proj.tile([128, 2, 512], F32, tag="proj", name="spin")

        def spin(n):
            for i in range(n):
                nc.tensor.matmul(
                    spin_ps[0:64, i % 2],
                    lhsT=ones64[:],
                    rhs=spin_sb[:],
                    start=True,
                    stop=True,
                )

        spin(12)

        def qkT_proj_unit(xT_sb, wqk_sb, jt, lh):
            """qkT[j, l] = sum_c wqk[c, j] xT[c, l] for one (j-tile, L-half)."""
            ps = ps_proj.tile([128, 2, 512], F32, tag="proj")
            for ct in range(CT):
                for lc in range(2):
                    nc.tensor.matmul(
                        ps[:, lc],
                        lhsT=wqk_sb[:, ct, jt * 128 : (jt + 1) * 128],
                        rhs=xT_sb[:, ct, lh * 1024 + lc * 512 : lh * 1024 + (lc + 1) * 512],
                        start=(ct == 0),
                        stop=(ct == CT - 1),
                    )
            nc.vector.tensor_scalar_add(
                qkT_sb[:, jt, 2 * lh : 2 * lh + 2, :], ps[:], bqk_sb[:, jt : jt + 1]
            )

        def score_kt(p, jq, expT, kt):
            """One k-tile of scoresT + its exp for head pair p, chunk jq.
            exp runs on ACT (true exp, scale fused) or on DVE (Schraudolph
            fused-multiply-add writing the bf16 bit pattern as int16)."""
            S = ps_s.tile([128, 2, 512], F32, tag="s")
            jl, off = kt // 4, (kt % 4) * 128
            nc.tensor.matmul(
                S[:, 0],
                lhsT=qkT_sb[0:64, 4 + p, jl, off : off + 128],
                rhs=qkT_sb[0:64, p, jq, :],
                start=True,
                stop=True,
            )
            nc.tensor.matmul(
                S[:, 1],
                lhsT=qkT_sb[64:128, 4 + p, jl, off : off + 128],
                rhs=qkT_sb[64:128, p, jq, :],
                start=True,
                stop=True,
            )
            if kt in DVE_KT:
                nc.vector.tensor_scalar(
                    expT[:, kt].bitcast(I16), S[:], SCH_A, SCH_B, MULT, ADD
                )
            else:
                nc.scalar.activation(expT[:, kt], S[:], FT.Exp, scale=SCALE)

        def av_alloc():
            avD = ps_avD.tile([128, 512], F32, tag="avD")
            avS = ps_avS.tile([128, 512], F32, tag="avS")
            return avD, avS

        def av_mms(avD, avS, p, jq, expT, kts):
            """AV accumulation-group matmuls: head A -> avD rows 0:64,
            head B -> avD rows 64:128 (concurrent col-tile pair)."""
            hA, hB = 2 * p, 2 * p + 1
            for kt in kts:
                st, sp = kt == 0, kt == KT - 1
                nc.tensor.matmul(
                    avD[0:64],
                    lhsT=V_sb[:, kt, hA * 64 : hA * 64 + 64],
                    rhs=expT[:, kt, 0],
                    start=st,
                    stop=sp,
                )
                nc.tensor.matmul(
                    avD[64:128],
                    lhsT=V_sb[:, kt, hB * 64 : hB * 64 + 64],
                    rhs=expT[:, kt, 1],
                    start=st,
                    stop=sp,
                )

        def sum_mms(avD, avS, expT, kts):
            """Row sums, replicated across 64 partitions (M=64 ones): head A
            -> avS rows 0:64, head B -> avS rows 64:128, partition-aligned
            with avD so the reciprocal/normalize are single whole-tile ops."""
            for kt in kts:
                st, sp = kt == 0, kt == KT - 1
                nc.tensor.matmul(
                    avS[0:64], lhsT=ones64[:], rhs=expT[:, kt, 0], start=st, stop=sp
                )
                nc.tensor.matmul(
                    avS[64:128], lhsT=ones64[:], rhs=expT[:, kt, 1], start=st, stop=sp
                )

        def norm_part(p, jq, avD, avS):
            # 1/sums via bit-trick seed + one Newton step (all [128,512], both
            # heads at once), then a single PSUM-side multiply into outT.
            r0 = r_pool.tile([128, 512], I32, tag="r0")
            nc.vector.tensor_scalar(
                r0[:], avS[:].bitcast(I32), -1.0, RCP_MAGIC, MULT, ADD
            )
            t_sb = r_pool.tile([128, 512], F32, tag="t")
            nc.vector.tensor_tensor(t_sb[:], avS[:], r0[:].bitcast(F32), MULT)
            u_sb = r_pool.tile([128, 512], F32, tag="u")
            nc.vector.tensor_scalar(u_sb[:], t_sb[:], -1.0, 2.0, MULT, ADD)
            r1 = r_pool.tile([128, 512], F32, tag="r1")
            nc.vector.tensor_tensor(r1[:], u_sb[:], r0[:].bitcast(F32), MULT)
            cols = slice(jq * 512, (jq + 1) * 512)
            nc.vector.tensor_tensor(outT_sb[:, p, cols], avD[:], r1[:], MULT)

        def y_unit_slices(lt, pool=None, tag="proj"):
            """y[l, f] = sum_d outT[d, l] wo[d, f] for one l-tile, split into
            two drippable half-slices (one per 512-wide f chunk), each with
            its own SBUF staging half + output DMA. Copies alternate DVE and
            ACT; DMAs alternate the gpsimd/sync queues. The last chunk routes
            psy through the (by then idle) score banks so consecutive l-tile
            groups pipeline instead of waiting on each other's copies."""
            box = {}
            pool = ps_proj if pool is None else pool

            def emit(fc, lt=lt):
                if fc == 0:
                    box["psy"] = pool.tile(
                        [128, 2, 512], F32, tag=tag, name=f"psy_{lt}"
                    )
                psy = box["psy"]
                y_sb = y_pool.tile([128, 512], BF16, tag="y", name=f"y_{lt}_{fc}")
                for p in range(NP):
                    nc.tensor.matmul(
                        psy[:, fc],
                        lhsT=outT_sb[:, p, lt * 128 : (lt + 1) * 128],
                        rhs=wo_sb[:, p, fc * 512 : (fc + 1) * 512],
                        start=(p == 0),
                        stop=(p == NP - 1),
                    )
                if fc == 0:
                    nc.vector.tensor_copy(y_sb[:], psy[:, fc])
                else:
                    nc.scalar.copy(y_sb[:], psy[:, fc])
                deng = nc.gpsimd if (lt + fc) % 2 == 0 else nc.sync
                deng.dma_start(
                    y_d[lt * 128 : (lt + 1) * 128, fc * 512 : (fc + 1) * 512],
                    y_sb[:],
                )

            return [lambda fc=fc: emit(fc) for fc in range(2)]

        with tc.tile_pool(name="xw", bufs=1) as xw_pool:
            # ct-interleaved input DMAs: the first projection units consume
            # c-tiles in order, so they start as soon as tile 0 lands.
            xT_sb = xw_pool.tile([128, CT, L], BF16)
            wqk_sb = xw_pool.tile([128, CT, 1024], BF16)
            xT_r = xT_d.rearrange("(t p) l -> p t l", p=128)
            wqk_r = wqk_d.rearrange("(t p) j -> p t j", p=128)
            # wqk rides the scalar queue so xT (the fatter stream) has the
            # sync queue to itself — inputs land ~5us sooner
            for ct in range(CT):
                nc.scalar.dma_start(wqk_sb[:, ct], wqk_r[:, ct])
                nc.sync.dma_start(xT_sb[:, ct], xT_r[:, ct])

            def V_proj_unit(wv_sb, lt):
                pool, tag = (ps_avD, "avD") if lt % 2 == 0 else (ps_avS, "avS")
                psv = pool.tile([128, 512], F32, tag=tag)
                for ct in range(CT):
                    nc.tensor.matmul(
                        psv[:],
                        lhsT=xT_sb[:, ct, lt * 128 : (lt + 1) * 128],
                        rhs=wv_sb[:, ct, :],
                        start=(ct == 0),
                        stop=(ct == CT - 1),
                    )
                nc.vector.tensor_copy(V_sb[:, lt, :], psv[:])

            def qkT_unit_slices(jt, lh):
                """A qkT projection unit split into 4 drippable slices of
                4 matmuls (the psum accumulation group spans the slices)."""
                box = {}

                def emit(i, jt=jt, lh=lh):
                    if i == 0:
                        box["ps"] = ps_proj.tile(
                            [128, 2, 512], F32, tag="proj", name=f"proj_{jt}_{lh}"
                        )
                    ps = box["ps"]
                    for ct in (2 * i, 2 * i + 1):
                        for lc in range(2):
                            nc.tensor.matmul(
                                ps[:, lc],
                                lhsT=wqk_sb[:, ct, jt * 128 : (jt + 1) * 128],
                                rhs=xT_sb[
                                    :,
                                    ct,
                                    lh * 1024 + lc * 512 : lh * 1024 + (lc + 1) * 512,
                                ],
                                start=(ct == 0),
                                stop=(ct == CT - 1),
                            )
                    if i == 3:
                        nc.vector.tensor_scalar_add(
                            qkT_sb[:, jt, 2 * lh : 2 * lh + 2, :],
                            ps[:],
                            bqk_sb[:, jt : jt + 1],
                        )

                return [lambda i=i: emit(i) for i in range(4)]

            # Emission schedule: per chunk c we emit its AV groups (paced by
            # its exps), then the first 4 score k-tiles of chunk c+1 woven
            # between the two halves of c's row-sum pass (the sums can only
            # start once the AV groups close, i.e. after c's last exp), then
            # the normalize, then the remaining score k-tiles of c+1 with
            # projection work dripped one slice per k-tile.
            with tc.tile_pool(name="wv", bufs=1) as wv_pool:
                wv_sb = wv_pool.tile([128, CT, 512], BF16)
                nc.scalar.dma_start(wv_sb[:], wv_d.rearrange("(t p) j -> p t j", p=128))

                # pair 0 q/k projections up front, interleaved per c-tile so
                # the PE stays dense while the input DMA streams. They ride
                # the (otherwise still idle) score PSUM banks so both can be
                # in flight at once.
                chunks = [(p, jq) for p in range(NP) for jq in range(JQ)]
                psA = ps_s.tile([128, 2, 512], F32, tag="s", name="projA")
                psB = ps_s.tile([128, 2, 512], F32, tag="s", name="projB")
                for ct in range(CT):
                    spin(2)
                    for ps, jt in ((psA, 4), (psB, 0)):
                        for lc in range(2):
                            nc.tensor.matmul(
                                ps[:, lc],
                                lhsT=wqk_sb[:, ct, jt * 128 : (jt + 1) * 128],
                                rhs=xT_sb[:, ct, lc * 512 : (lc + 1) * 512],
                                start=(ct == 0),
                                stop=(ct == CT - 1),
                            )
                for ps, jt in ((psA, 4), (psB, 0)):
                    nc.vector.tensor_scalar_add(
                        qkT_sb[:, jt, 0:2, :], ps[:], bqk_sb[:, jt : jt + 1]
                    )
                exp0 = exp_pool.tile([128, KT, 2, 512], BF16, tag="expT")
                for kt in range(8):
                    score_kt(0, 0, exp0, kt)
                    spin(2)
                qkT_proj_unit(xT_sb, wqk_sb, 4, 1)
                for kt in range(8, KT):
                    score_kt(0, 0, exp0, kt)
                    spin(1)
                # chunk (0,1) scores with the V projection dripped per k-tile
                exp1 = exp_pool.tile([128, KT, 2, 512], BF16, tag="expT")
                for kt in range(KT):
                    score_kt(0, 1, exp1, kt)
                    V_proj_unit(wv_sb, kt)
                qkT_proj_unit(xT_sb, wqk_sb, 0, 1)

            exps = {0: exp0, 1: exp1}
            for ci in range(len(chunks)):
                p, jq = chunks[ci]
                nxt = chunks[ci + 1] if ci + 1 < len(chunks) else None
                emit_nxt = nxt is not None and (ci + 1) not in exps
                if emit_nxt:
                    exps[ci + 1] = exp_pool.tile([128, KT, 2, 512], BF16, tag="expT", name=f"expT_{ci+1}")
                expT = exps.pop(ci)
                avD, avS = av_alloc()
                av_mms(avD, avS, p, jq, expT, range(0, KT - 1))
                if emit_nxt:
                    # runs during this chunk's last exp (S slot frees at kt14)
                    score_kt(*nxt, exps[ci + 1], 0)
                av_mms(avD, avS, p, jq, expT, [KT - 1])
                if emit_nxt:
                    score_kt(*nxt, exps[ci + 1], 1)
                sum_mms(avD, avS, expT, range(0, 6))
                if emit_nxt:
                    score_kt(*nxt, exps[ci + 1], 2)
                sum_mms(avD, avS, expT, range(6, 11))
                if emit_nxt:
                    score_kt(*nxt, exps[ci + 1], 3)
                sum_mms(avD, avS, expT, range(11, KT))
                if emit_nxt:
                    score_kt(*nxt, exps[ci + 1], 4)
                norm_part(p, jq, avD, avS)

                # filler: next pair's projections (pairs 0-2) or the
                # output projection (pair 3), dripped per score k-tile
                if p < NP - 1:
                    nj = p + 1
                    jt, lh = [(4 + nj, 0), (4 + nj, 1), (nj, 0), (nj, 1)][jq]
                    drip = qkT_unit_slices(jt, lh)
                else:
                    drip = []
                    last = ci == len(chunks) - 1
                    for lt in range(4 * jq, 4 * jq + 4):
                        if last and lt < 4 * jq + 2:
                            drip.extend(y_unit_slices(lt, pool=ps_s, tag="s"))
                        else:
                            drip.extend(y_unit_slices(lt))
                if nxt is None:
                    # final chunk: no next-chunk scores to fill the PE while
                    # the norm chain runs on DVE — spin so the HAM clock gate
                    # stays released for the closing y-projection burst
                    spin2 = ps_proj.tile([128, 2, 512], F32, tag="proj", name="spin2")
                    for i in range(14):
                        nc.tensor.matmul(
                            spin2[0:64, i % 2],
                            lhsT=ones64[:],
                            rhs=spin_sb[:],
                            start=True,
                            stop=True,
                        )
                for kt in range(5, KT):
                    if emit_nxt:
                        score_kt(*nxt, exps[ci + 1], kt)
                    if drip:
                        drip.pop(0)()
                while drip:
                    drip.pop(0)()

    nc.finalize()
    return nc


_NC_CACHE = None


def _get_program():
    global _NC_CACHE
    if _NC_CACHE is None:
        _NC_CACHE = _build_program()
    return _NC_CACHE


def _make_in_maps(x, W_in, b_in, W_out):
    bf = ml_dtypes.bfloat16
    in_maps = []
    for c in range(NCORES):
        n, g = c // 2, c % 2
        h0 = g * HG  # first global head
        j0 = h0 * D  # 512*g
        xT = np.ascontiguousarray(x[n].T).astype(bf)  # [C, L]
        wqk = np.concatenate(
            [W_in[:, j0 : j0 + 512], W_in[:, QKV + j0 : QKV + j0 + 512]], axis=1
        ).astype(bf)
        wv = np.ascontiguousarray(W_in[:, 2 * QKV + j0 : 2 * QKV + j0 + 512]).astype(bf)
        wo = np.ascontiguousarray(W_out[j0 : j0 + 512, :]).astype(bf)
        bqk = (
            np.concatenate([b_in[j0 : j0 + 512], b_in[QKV + j0 : QKV + j0 + 512]])
            .astype(np.float32)
            .reshape(8, 128)
            .T.copy()
        )
        in_maps.append({"xT": xT, "wqk": wqk, "wv": wv, "wo": wo, "bqk": bqk})
    return in_maps


def kernel(x, W_in, b_in, W_out, b_out):
    global LAST_RESULTS
    x = np.asarray(x, dtype=np.float32)
    W_in = np.asarray(W_in, dtype=np.float32)
    b_in = np.asarray(b_in, dtype=np.float32)
    W_out = np.asarray(W_out, dtype=np.float32)
    b_out = np.asarray(b_out, dtype=np.float32)

    nc = _get_program()
    in_maps = _make_in_maps(x, W_in, b_in, W_out)
    res = run_bass_kernel_spmd(nc, in_maps, list(range(NCORES)), trace=TRACE)
    LAST_RESULTS = res

    # host bias: b_out + b_v @ W_out  (b_v enters linearly through the
    # softmax-normalized value average: A@(V+b_v) = A@V + b_v)
    host_bias = (
        b_out.astype(np.float64)
        + b_in[2 * QKV :].astype(np.float64) @ W_out.astype(np.float64)
    ).astype(np.float32)

    out = np.empty((N, L, F), dtype=np.float32)
    for n in range(N):
        y0 = np.asarray(res.results[2 * n]["y"], dtype=np.float32)
        y1 = np.asarray(res.results[2 * n + 1]["y"], dtype=np.float32)
        out[n] = y0 + y1 + host_bias
    return out
